# revision 3
# baseline (speedup 1.0000x reference)
"""ChebConv-with-spatial-attention Trainium2 kernel (8 NeuronCores, SPMD data-parallel).

Math (per batch b), using cheb[0] == I (Chebyshev T0) and the exact fold
E = cheb[2] + I (so the identity parts never touch the device):

    out = relu( (att*I) x (Th0 - Th2)            # diagonal path, bf16
              + (att*C1) x Th1                   # fp8 masked matmul
              + (att*E)  x Th2 )                 # fp8 masked matmul
    with the diag(E) part of the last term folded into a bf16 correction
    (added during the PSUM->SBUF copy), so only SMALL off-diagonal values
    ever go through fp8.

Device mapping (per core, 2 batches):
    stage 1: R_k^T[tf, i] = sum_j xh[j, tf] * mask_k[j, i] via fp8
             DoubleRow matmuls (contraction 256 = 2 j-chunks per PE pass).
    stage 2: out[i, (t,o)] += lhs_k[tf_blk, i]^T @ thetap[k] over k, tf_blk
             in bf16 with block-diagonal padded Theta; fused ReLU+bf16 on
             copy-out.

Host pre-processing: premultiplied fp8 masks (power-of-2 scaled; scales
compensated inside thetap), fp8-swizzled xh, bf16 xht = attd*x (transposed),
diag(E) broadcast vector, block-diag thetap.
"""

import numpy as np

B, T, N, F_IN, F_OUT, K = 16, 12, 1024, 32, 64, 3
M_CORES = 8
NB = B // M_CORES          # batches per core
KM = 2                     # masked (non-diagonal) cheb paths: C1, E_off
P = 128                    # SBUF partitions
NJ = N // P                # 8 contraction chunks
NPAIR = NJ // 2            # 4 DoubleRow chunk pairs
TF = T * F_IN              # 384
NTFB = TF // P             # 3 tf blocks
TBLK = P // F_IN           # 4 t's per tf block
IS = 512                   # stage-1 strip width (1 PSUM bank of fp32)
NIS = N // IS              # 2 i strips
TO = TBLK * F_OUT          # 256 = stage-2 rhs width

_cache = {}


def _build(reps=1):
    import concourse.bacc as bacc
    import concourse.mybir as mybir
    import concourse.tile as tile

    DT = mybir.dt.bfloat16
    DT8 = mybir.dt.float8e4
    DTF = mybir.dt.float32
    DR = mybir.MatmulPerfMode.DoubleRow

    nc = bacc.Bacc("TRN2", target_bir_lowering=False, debug=False)
    mask_d = nc.dram_tensor("mask8", [NB, KM, N, N], DT8, kind="ExternalInput")
    xh_d = nc.dram_tensor("xh8", [NB, P, NJ * TF], DT8, kind="ExternalInput")
    xht_d = nc.dram_tensor("xht", [NB, TF, N], DT, kind="ExternalInput")
    vec_d = nc.dram_tensor("gvec", [P, N], DT, kind="ExternalInput")
    thp_d = nc.dram_tensor("thetap", [K, P, TO], DT, kind="ExternalInput")
    # [b, i, t, o] layout: device stores are fully contiguous; host permutes
    # back to [b, t, i, o] afterwards.
    out_d = nc.dram_tensor("out", [NB, N, T, F_OUT], DT, kind="ExternalOutput")

    with tile.TileContext(nc) as tc:
        with (
            tc.tile_pool(name="mask", bufs=2 * KM) as mask_pool,
            tc.tile_pool(name="xh", bufs=2) as xh_pool,
            tc.tile_pool(name="xht", bufs=2) as xht_pool,
            tc.tile_pool(name="x2s", bufs=2) as x2s_pool,
            tc.tile_pool(name="rt", bufs=2) as rt_pool,
            tc.tile_pool(name="wts", bufs=1) as wts_pool,
            tc.tile_pool(name="osb", bufs=3) as out_pool,
            tc.tile_pool(name="rtps", bufs=2, space="PSUM") as rtps_pool,
            tc.tile_pool(name="outps", bufs=2, space="PSUM") as outps_pool,
        ):
            thp_sb = wts_pool.tile([P, K * TO], DT, tag="thp")
            vec_sb = wts_pool.tile([P, N], DT, tag="gvec")

            for rep in range(reps):
                first = rep == 0
                mask_tiles = {}
                xh_tiles = {}
                xht_tiles = {}

                def load_b(b, split):
                    # all input loads on SP: its SEQ does nothing else, so
                    # issue order == transfer order (HWDGE+DMA are in-order).
                    xh_sb = xh_pool.tile([P, NJ * TF], DT8, tag="xh", name="xh_sb")
                    nc.sync.dma_start(xh_sb[:], xh_d.ap()[b])
                    xh_tiles[b] = xh_sb
                    for k in range(KM):
                        m_sb = mask_pool.tile(
                            [P, NJ * N], DT8, tag=f"mask{b}{k}", name=f"m{b}{k}"
                        )
                        msrc = mask_d.ap()[b][k].rearrange("(jb p) i -> p jb i", p=P)
                        mdst = m_sb[:].rearrange("p (jb i) -> p jb i", jb=NJ)
                        nch = split if k == 0 else 2
                        step = NJ // nch
                        for c in range(nch):
                            nc.sync.dma_start(
                                mdst[:, c * step : (c + 1) * step, :],
                                msrc[:, c * step : (c + 1) * step, :],
                            )
                        mask_tiles[b, k] = m_sb
                    xht_sb = xht_pool.tile([P, NTFB * N], DT, tag="xht", name="xht_sb")
                    nc.sync.dma_start(
                        xht_sb[:].rearrange("p (c i) -> p c i", c=NTFB),
                        xht_d.ap()[b].rearrange("(c p) i -> p c i", p=P),
                    )
                    xht_tiles[b] = xht_sb

                load_b(0, split=4)
                if first:
                    nc.sync.dma_start(vec_sb[:], vec_d.ap())
                    nc.sync.dma_start(
                        thp_sb[:].rearrange("p (k n) -> p k n", k=K),
                        thp_d.ap().rearrange("k p n -> p k n"),
                    )
                load_b(1, split=2)

                rt_tiles = {}
                x2s_tiles = {}
                # stage 1: R^T in bf16, fp8 DoubleRow matmuls (contraction 256)
                for b in range(NB):
                    xh3 = xh_tiles[b][:].rearrange("p (jb t) -> p jb t", jb=NJ)
                    # diag(E) correction operand: xht * gvec, same [tf, i]
                    # layout as the stage-1 PSUM result.
                    x2s_sb = x2s_pool.tile([P, NTFB * N], DT, tag="x2s", name="x2s_sb")
                    for c in range(NTFB):
                        nc.vector.tensor_mul(
                            x2s_sb[:, c * N : (c + 1) * N],
                            xht_tiles[b][:, c * N : (c + 1) * N],
                            vec_sb[:],
                        )
                    x2s_tiles[b] = x2s_sb
                    rt_sb = rt_pool.tile([P, KM * NTFB * N], DT, tag="rt")
                    rt_tiles[b] = rt_sb
                    for k in range(KM):
                        m3 = mask_tiles[b, k][:].rearrange(
                            "p (jb i) -> p jb i", jb=NJ
                        )
                        for tfb in range(NTFB):
                            rtps = rtps_pool.tile([P, N], DTF, tag="rtps", name="rtps")
                            for q in range(NPAIR):
                                lhs = xh3[:, 2 * q : 2 * q + 2, tfb * P : (tfb + 1) * P]
                                for s in range(NIS):
                                    nc.tensor.matmul(
                                        rtps[:, s * IS : (s + 1) * IS],
                                        lhs,
                                        m3[:, 2 * q : 2 * q + 2, s * IS : (s + 1) * IS],
                                        start=(q == 0),
                                        stop=(q == NPAIR - 1),
                                        perf_mode=DR,
                                    )
                            base = (k * NTFB + tfb) * N
                            if k == 0:
                                nc.scalar.copy(rt_sb[:, base : base + N], rtps[:])
                            else:
                                # k=1 is the E path: add the bf16 diag(E)
                                # correction while copying out of PSUM.
                                nc.vector.tensor_add(
                                    rt_sb[:, base : base + N],
                                    rtps[:],
                                    x2s_sb[:, tfb * N : (tfb + 1) * N],
                                )

                # stage 2: out[i, (t,o)] over k in {diag, C1, E} per tf block
                for b in range(NB):
                    rt_sb = rt_tiles[b]
                    xht_sb = xht_tiles[b]
                    for ic in range(NJ):
                        ops = outps_pool.tile([P, NTFB * TO], DTF, tag="ops", name="ops")
                        for tfb in range(NTFB):
                            for k in range(K):
                                if k == 0:
                                    lhs2 = xht_sb[:, tfb * N + ic * P : tfb * N + ic * P + P]
                                else:
                                    base = ((k - 1) * NTFB + tfb) * N + ic * P
                                    lhs2 = rt_sb[:, base : base + P]
                                nc.tensor.matmul(
                                    ops[:, tfb * TO : (tfb + 1) * TO],
                                    lhs2,
                                    thp_sb[:, k * TO : (k + 1) * TO],
                                    start=(k == 0),
                                    stop=(k == K - 1),
                                )
                        osb = out_pool.tile([P, T * F_OUT], DT, tag="osb")
                        if ic % 2 == 0:
                            nc.scalar.activation(
                                osb[:], ops[:], mybir.ActivationFunctionType.Relu
                            )
                        else:
                            nc.vector.tensor_relu(osb[:], ops[:])
                        # stores ride the Pool SWDGE queue: they wait on
                        # compute, so they must not block SP's load stream.
                        nc.gpsimd.dma_start(
                            out_d.ap()[b][ic * P : (ic + 1) * P, :, :],
                            osb[:].rearrange("p (t o) -> p t o", t=T),
                        )

    nc.compile()
    return nc


def _prep(x, att, cheb, Theta):
    """Host-side packing shared by kernel() and test harnesses.

    Returns (in_maps, unscale) where in_maps is the per-core ExternalInput
    dict list.
    """
    from ml_dtypes import bfloat16, float8_e4m3

    x = np.asarray(x, dtype=np.float32)
    att = np.asarray(att, dtype=np.float32)
    cheb = np.asarray(cheb, dtype=np.float32)
    Theta = np.asarray(Theta, dtype=np.float32)

    eye = np.eye(N, dtype=np.float32)
    C1 = cheb[1]
    E = cheb[2] + eye
    e = np.diagonal(E).copy()
    Eoff = E - np.diag(e)

    def pow2_scale(target, cur):
        return float(2.0 ** np.floor(np.log2(target / max(cur, 1e-30))))

    # premultiplied fp8 masks, transposed for stage 1 ([j, i] layout)
    m1 = C1[None, :, :] * att          # [B, i, j]
    m2 = Eoff[None, :, :] * att
    s1 = pow2_scale(100.0, np.abs(m1).max())
    s2 = pow2_scale(100.0, np.abs(m2).max())
    sx = pow2_scale(100.0, np.abs(x).max())
    mask8 = np.empty((B, KM, N, N), dtype=float8_e4m3)
    mask8[:, 0] = (m1 * s1).transpose(0, 2, 1).astype(float8_e4m3)
    mask8[:, 1] = (m2 * s2).transpose(0, 2, 1).astype(float8_e4m3)

    # xh fp8 pre-swizzled so each SBUF partition's row is contiguous in DRAM:
    # xh8[b, p, jb*TF + tf] = x[b, t, jb*128+p, f] * sx
    xh = (x * sx).transpose(0, 2, 1, 3).reshape(B, N, TF)
    xh8 = np.ascontiguousarray(
        xh.reshape(B, NJ, P, TF).transpose(0, 2, 1, 3).reshape(B, P, NJ * TF)
    ).astype(float8_e4m3)

    # diagonal path: xht[b, tf, i] = x[b,t,i,f] * att[b,i,i], bf16
    attd = np.einsum("bii->bi", att)
    xht = np.ascontiguousarray(
        (x * attd[:, None, :, None]).transpose(0, 1, 3, 2).reshape(B, TF, N)
    ).astype(bfloat16)

    # diag(E) broadcast vector in stage-1 scaled units
    gvec = np.broadcast_to((e * s2 * sx).astype(bfloat16)[None, :], (P, N))
    gvec = np.ascontiguousarray(gvec)

    # block-diag thetap; per-slot scale compensation for the fp8 paths
    Th = np.stack([Theta[0] - Theta[2], Theta[1] / (s1 * sx), Theta[2] / (s2 * sx)])
    thetap = np.zeros((K, P, TO), dtype=np.float32)
    for tr in range(TBLK):
        thetap[:, tr * F_IN : (tr + 1) * F_IN, tr * F_OUT : (tr + 1) * F_OUT] = Th
    thetap = thetap.astype(bfloat16)

    in_maps = [
        {
            "mask8": mask8[c * NB : (c + 1) * NB],
            "xh8": xh8[c * NB : (c + 1) * NB],
            "xht": xht[c * NB : (c + 1) * NB],
            "gvec": gvec,
            "thetap": thetap,
        }
        for c in range(M_CORES)
    ]
    return in_maps


def kernel(x, spatial_attention, cheb, Theta):
    from concourse.bass_utils import run_bass_kernel_spmd

    cheb = np.asarray(cheb, dtype=np.float32)
    if np.abs(cheb[0] - np.eye(N, dtype=np.float32)).max() > 1e-5:
        # T0 of any Chebyshev basis is the identity; the device kernel
        # hardcodes that. Unreachable for this problem's generator — kept
        # only so a different basis still returns the right answer.
        x = np.asarray(x, dtype=np.float32)
        att = np.asarray(spatial_attention, dtype=np.float32)
        Theta = np.asarray(Theta, dtype=np.float32)
        tk = cheb[None] * att[:, None]
        rhs = np.einsum("bkij,btjf->btkif", tk, x)
        return np.maximum(np.einsum("btkif,kfo->btio", rhs, Theta), 0.0)

    if "k" not in _cache:
        _cache["k"] = _build()
    nc = _cache["k"]

    in_maps = _prep(x, spatial_attention, cheb, Theta)
    try:
        res = run_bass_kernel_spmd(nc, in_maps, list(range(M_CORES)))
    except Exception:
        # transient NRT device hiccups recover on redispatch
        res = run_bass_kernel_spmd(nc, in_maps, list(range(M_CORES)))
    out = np.concatenate(
        [np.asarray(res.results[c]["out"]) for c in range(M_CORES)], axis=0
    )
    # device layout is [b, i, t, o] bf16 -> [b, t, i, o] fp32
    return np.ascontiguousarray(out.transpose(0, 2, 1, 3)).astype(np.float32)


# revision 4
# speedup vs baseline: 1.0409x; 1.0409x over previous
"""ChebConv-with-spatial-attention Trainium2 kernel (8 NeuronCores, SPMD data-parallel).

Math (per batch b), using cheb[0] == I (Chebyshev T0) and the exact fold
E = cheb[2] + I (so the identity parts never touch the device):

    out = relu( (att*I) x (Th0 - Th2)            # diagonal path, bf16
              + (att*C1) x Th1                   # fp8 masked matmul
              + (att*E)  x Th2 )                 # fp8 masked matmul
    with the diag(E) part of the last term folded into a bf16 correction
    (added during the PSUM->SBUF copy), so only SMALL off-diagonal values
    ever go through fp8.

Device mapping (per core, 2 batches):
    stage 1: R_k^T[tf, i] = sum_j xh[j, tf] * mask_k[j, i] via fp8
             DoubleRow matmuls (contraction 256 = 2 j-chunks per PE pass).
    stage 2: out[i, (t,o)] += lhs_k[tf_blk, i]^T @ thetap[k] over k, tf_blk
             in bf16 with block-diagonal padded Theta; ReLU on copy-out
             (split across Act+DVE so PSUM drains faster than PE refills).

Schedule: loads stream on SP in consumption order; per batch the kernel
runs stage-1 then stage-2 so b0's stage-2 covers b1's mask loads. Output
stores ride Pool's SWDGE queue for b0 and SP for b1 (SP's load stream is
done by then), two i-chunks per store.

Host pre-processing: premultiplied fp8 masks (power-of-2 scaled; scales
compensated inside thetap), fp8-swizzled xh, bf16 xht = attd*x (transposed),
diag(E) broadcast vector, block-diag thetap.
"""

import numpy as np

B, T, N, F_IN, F_OUT, K = 16, 12, 1024, 32, 64, 3
M_CORES = 8
NB = B // M_CORES          # batches per core
KM = 2                     # masked (non-diagonal) cheb paths: C1, E_off
P = 128                    # SBUF partitions
NJ = N // P                # 8 contraction chunks
NPAIR = NJ // 2            # 4 DoubleRow chunk pairs
TF = T * F_IN              # 384
NTFB = TF // P             # 3 tf blocks
TBLK = P // F_IN           # 4 t's per tf block
IS = 512                   # stage-1 strip width (1 PSUM bank of fp32)
NIS = N // IS              # 2 i strips
TO = TBLK * F_OUT          # 256 = stage-2 rhs width
TFO = T * F_OUT            # 768

_cache = {}


def _build(reps=1):
    import concourse.bacc as bacc
    import concourse.mybir as mybir
    import concourse.tile as tile

    DT = mybir.dt.bfloat16
    DT8 = mybir.dt.float8e4
    DTF = mybir.dt.float32
    DR = mybir.MatmulPerfMode.DoubleRow
    Relu = mybir.ActivationFunctionType.Relu

    nc = bacc.Bacc("TRN2", target_bir_lowering=False, debug=False)
    mask_d = nc.dram_tensor("mask8", [NB, KM, N, N], DT8, kind="ExternalInput")
    xh_d = nc.dram_tensor("xh8", [NB, P, NJ * TF], DT8, kind="ExternalInput")
    xht_d = nc.dram_tensor("xht", [NB, TF, N], DT, kind="ExternalInput")
    vec_d = nc.dram_tensor("gvec", [P, N], DT, kind="ExternalInput")
    thp_d = nc.dram_tensor("thetap", [K, P, TO], DT, kind="ExternalInput")
    # [b, i, t, o] layout: device stores are fully contiguous; host permutes
    # back to [b, t, i, o] afterwards.
    out_d = nc.dram_tensor("out", [NB, N, T, F_OUT], DT, kind="ExternalOutput")

    with tile.TileContext(nc) as tc:
        with (
            tc.tile_pool(name="mask", bufs=2 * KM) as mask_pool,
            tc.tile_pool(name="xh", bufs=2) as xh_pool,
            tc.tile_pool(name="xht", bufs=2) as xht_pool,
            tc.tile_pool(name="x2s", bufs=2) as x2s_pool,
            tc.tile_pool(name="rt", bufs=2) as rt_pool,
            tc.tile_pool(name="wts", bufs=1) as wts_pool,
            tc.tile_pool(name="osb", bufs=3) as out_pool,
            tc.tile_pool(name="rtps", bufs=2, space="PSUM") as rtps_pool,
            tc.tile_pool(name="outps", bufs=2, space="PSUM") as outps_pool,
        ):
            thp_sb = wts_pool.tile([P, K * TO], DT, tag="thp")
            vec_sb = wts_pool.tile([P, N], DT, tag="gvec")

            for rep in range(reps):
                first = rep == 0
                mask_tiles = {}
                xh_tiles = {}
                xht_tiles = {}

                def load_b(b):
                    # all input loads on SP in consumption order: its SEQ does
                    # nothing else, so issue order == transfer order.
                    xh_sb = xh_pool.tile([P, NJ * TF], DT8, tag="xh", name="xh_sb")
                    xh3d = xh_sb[:].rearrange("p (jb t) -> p jb t", jb=NJ)
                    xsrc = xh_d.ap()[b].rearrange("p (jb t) -> p jb t", jb=NJ)
                    nxh = 2 if b == 0 else 1
                    for c in range(nxh):
                        h = NJ // nxh
                        nc.sync.dma_start(
                            xh3d[:, c * h : (c + 1) * h, :],
                            xsrc[:, c * h : (c + 1) * h, :],
                        )
                    xh_tiles[b] = xh_sb

                    def load_mask(k, nch):
                        m_sb = mask_pool.tile(
                            [P, NJ * N], DT8, tag=f"mask{b}{k}", name=f"m{b}{k}"
                        )
                        msrc = mask_d.ap()[b][k].rearrange("(jb p) i -> p jb i", p=P)
                        mdst = m_sb[:].rearrange("p (jb i) -> p jb i", jb=NJ)
                        step = NJ // nch
                        for c in range(nch):
                            nc.sync.dma_start(
                                mdst[:, c * step : (c + 1) * step, :],
                                msrc[:, c * step : (c + 1) * step, :],
                            )
                        mask_tiles[b, k] = m_sb

                    load_mask(0, 4 if b == 0 else 2)
                    xht_sb = xht_pool.tile([P, NTFB * N], DT, tag="xht", name="xht_sb")
                    nc.sync.dma_start(
                        xht_sb[:].rearrange("p (c i) -> p c i", c=NTFB),
                        xht_d.ap()[b].rearrange("(c p) i -> p c i", p=P),
                    )
                    xht_tiles[b] = xht_sb
                    load_mask(1, 2)
                    if b == 0 and first:
                        nc.sync.dma_start(vec_sb[:], vec_d.ap())
                        nc.sync.dma_start(
                            thp_sb[:].rearrange("p (k n) -> p k n", k=K),
                            thp_d.ap().rearrange("k p n -> p k n"),
                        )

                load_b(0)
                load_b(1)

                for b in range(NB):
                    # ---- stage 1: R^T bf16 via fp8 DoubleRow matmuls ----
                    xh3 = xh_tiles[b][:].rearrange("p (jb t) -> p jb t", jb=NJ)
                    # diag(E) correction operand: xht * gvec, same [tf, i]
                    # layout as the stage-1 PSUM result.
                    x2s_sb = x2s_pool.tile([P, NTFB * N], DT, tag="x2s", name="x2s_sb")
                    for c in range(NTFB):
                        nc.vector.tensor_mul(
                            x2s_sb[:, c * N : (c + 1) * N],
                            xht_tiles[b][:, c * N : (c + 1) * N],
                            vec_sb[:],
                        )
                    rt_sb = rt_pool.tile([P, KM * NTFB * N], DT, tag="rt")
                    for k in range(KM):
                        m3 = mask_tiles[b, k][:].rearrange(
                            "p (jb i) -> p jb i", jb=NJ
                        )
                        for tfb in range(NTFB):
                            rtps = rtps_pool.tile([P, N], DTF, tag="rtps", name="rtps")
                            for q in range(NPAIR):
                                lhs = xh3[:, 2 * q : 2 * q + 2, tfb * P : (tfb + 1) * P]
                                for s in range(NIS):
                                    nc.tensor.matmul(
                                        rtps[:, s * IS : (s + 1) * IS],
                                        lhs,
                                        m3[:, 2 * q : 2 * q + 2, s * IS : (s + 1) * IS],
                                        start=(q == 0),
                                        stop=(q == NPAIR - 1),
                                        perf_mode=DR,
                                    )
                            base = (k * NTFB + tfb) * N
                            if k == 0:
                                nc.scalar.copy(rt_sb[:, base : base + N], rtps[:])
                            else:
                                # k=1 is the E path: add the bf16 diag(E)
                                # correction while copying out of PSUM.
                                nc.vector.tensor_add(
                                    rt_sb[:, base : base + N],
                                    rtps[:],
                                    x2s_sb[:, tfb * N : (tfb + 1) * N],
                                )

                    # ---- stage 2: out[i, (t,o)] over {diag, C1, E} ----
                    xht_sb = xht_tiles[b]
                    osb = None
                    for ic in range(NJ):
                        ops = outps_pool.tile([P, NTFB * TO], DTF, tag="ops", name="ops")
                        for tfb in range(NTFB):
                            for k in range(K):
                                if k == 0:
                                    lhs2 = xht_sb[:, tfb * N + ic * P : tfb * N + ic * P + P]
                                else:
                                    base = ((k - 1) * NTFB + tfb) * N + ic * P
                                    lhs2 = rt_sb[:, base : base + P]
                                nc.tensor.matmul(
                                    ops[:, tfb * TO : (tfb + 1) * TO],
                                    lhs2,
                                    thp_sb[:, k * TO : (k + 1) * TO],
                                    start=(k == 0),
                                    stop=(k == K - 1),
                                )
                        # ReLU split across both copy engines so the PSUM
                        # tile frees faster than PE refills it.
                        if ic % 2 == 0:
                            osb = out_pool.tile([P, 2 * TFO], DT, tag="osb")
                        half = (ic % 2) * TFO
                        nc.scalar.activation(
                            osb[:, half : half + TFO // 2], ops[:, : TFO // 2], Relu
                        )
                        nc.vector.tensor_relu(
                            osb[:, half + TFO // 2 : half + TFO], ops[:, TFO // 2 :]
                        )
                        if ic % 2 == 1:
                            # two i-chunks per store; b0 on Pool's SWDGE
                            # queue, b1 on SP (its load stream is done).
                            eng = nc.gpsimd if b == 0 else nc.sync
                            pr = ic // 2
                            eng.dma_start(
                                out_d.ap()[b][pr * 2 * P : (pr + 1) * 2 * P, :, :]
                                .rearrange("(c p) t o -> p c (t o)", p=P),
                                osb[:].rearrange("p (c w) -> p c w", c=2),
                            )

    nc.compile()
    return nc


def _prep(x, att, cheb, Theta):
    """Host-side packing shared by kernel() and test harnesses.

    Returns the per-core ExternalInput dict list.
    """
    from ml_dtypes import bfloat16, float8_e4m3

    x = np.asarray(x, dtype=np.float32)
    att = np.asarray(att, dtype=np.float32)
    cheb = np.asarray(cheb, dtype=np.float32)
    Theta = np.asarray(Theta, dtype=np.float32)

    eye = np.eye(N, dtype=np.float32)
    C1 = cheb[1]
    E = cheb[2] + eye
    e = np.diagonal(E).copy()
    Eoff = E - np.diag(e)

    def pow2_scale(target, cur):
        return float(2.0 ** np.floor(np.log2(target / max(cur, 1e-30))))

    # premultiplied fp8 masks, transposed for stage 1 ([j, i] layout)
    m1 = C1[None, :, :] * att          # [B, i, j]
    m2 = Eoff[None, :, :] * att
    s1 = pow2_scale(100.0, np.abs(m1).max())
    s2 = pow2_scale(100.0, np.abs(m2).max())
    sx = pow2_scale(100.0, np.abs(x).max())
    mask8 = np.empty((B, KM, N, N), dtype=float8_e4m3)
    mask8[:, 0] = (m1 * s1).transpose(0, 2, 1).astype(float8_e4m3)
    mask8[:, 1] = (m2 * s2).transpose(0, 2, 1).astype(float8_e4m3)

    # xh fp8 pre-swizzled so each SBUF partition's row is contiguous in DRAM:
    # xh8[b, p, jb*TF + tf] = x[b, t, jb*128+p, f] * sx
    xh = (x * sx).transpose(0, 2, 1, 3).reshape(B, N, TF)
    xh8 = np.ascontiguousarray(
        xh.reshape(B, NJ, P, TF).transpose(0, 2, 1, 3).reshape(B, P, NJ * TF)
    ).astype(float8_e4m3)

    # diagonal path: xht[b, tf, i] = x[b,t,i,f] * att[b,i,i], bf16
    attd = np.einsum("bii->bi", att)
    xht = np.ascontiguousarray(
        (x * attd[:, None, :, None]).transpose(0, 1, 3, 2).reshape(B, TF, N)
    ).astype(bfloat16)

    # diag(E) broadcast vector in stage-1 scaled units
    gvec = np.broadcast_to((e * s2 * sx).astype(bfloat16)[None, :], (P, N))
    gvec = np.ascontiguousarray(gvec)

    # block-diag thetap; per-slot scale compensation for the fp8 paths
    Th = np.stack([Theta[0] - Theta[2], Theta[1] / (s1 * sx), Theta[2] / (s2 * sx)])
    thetap = np.zeros((K, P, TO), dtype=np.float32)
    for tr in range(TBLK):
        thetap[:, tr * F_IN : (tr + 1) * F_IN, tr * F_OUT : (tr + 1) * F_OUT] = Th
    thetap = thetap.astype(bfloat16)

    return [
        {
            "mask8": mask8[c * NB : (c + 1) * NB],
            "xh8": xh8[c * NB : (c + 1) * NB],
            "xht": xht[c * NB : (c + 1) * NB],
            "gvec": gvec,
            "thetap": thetap,
        }
        for c in range(M_CORES)
    ]


def kernel(x, spatial_attention, cheb, Theta):
    from concourse.bass_utils import run_bass_kernel_spmd

    cheb = np.asarray(cheb, dtype=np.float32)
    if np.abs(cheb[0] - np.eye(N, dtype=np.float32)).max() > 1e-5:
        # T0 of any Chebyshev basis is the identity; the device kernel
        # hardcodes that. Unreachable for this problem's generator — kept
        # only so a different basis still returns the right answer.
        x = np.asarray(x, dtype=np.float32)
        att = np.asarray(spatial_attention, dtype=np.float32)
        Theta = np.asarray(Theta, dtype=np.float32)
        tk = cheb[None] * att[:, None]
        rhs = np.einsum("bkij,btjf->btkif", tk, x)
        return np.maximum(np.einsum("btkif,kfo->btio", rhs, Theta), 0.0)

    if "k" not in _cache:
        _cache["k"] = _build()
    nc = _cache["k"]

    in_maps = _prep(x, spatial_attention, cheb, Theta)
    try:
        res = run_bass_kernel_spmd(nc, in_maps, list(range(M_CORES)))
    except Exception:
        # transient NRT device hiccups recover on redispatch
        res = run_bass_kernel_spmd(nc, in_maps, list(range(M_CORES)))
    out = np.concatenate(
        [np.asarray(res.results[c]["out"]) for c in range(M_CORES)], axis=0
    )
    # device layout is [b, i, t, o] bf16 -> [b, t, i, o] fp32
    return np.ascontiguousarray(out.transpose(0, 2, 1, 3)).astype(np.float32)


# revision 7
# speedup vs baseline: 1.0710x; 1.0289x over previous
"""ChebConv-with-spatial-attention Trainium2 kernel (8 NeuronCores, SPMD data-parallel).

Math (per batch b), using cheb[0] == I (Chebyshev T0) and the exact fold
E = cheb[2] + I (so the identity parts never touch the device):

    out = relu( (att*I) x (Th0 - Th2)            # diagonal path, bf16
              + (att*C1) x Th1                   # fp8 masked matmul
              + (att*E)  x Th2 )                 # fp8 masked matmul
    with the diag(E) part of the last term folded into a bf16 correction
    (added during the PSUM->SBUF copy), so only SMALL off-diagonal values
    ever go through fp8.

Device mapping (per core, 2 batches):
    stage 1: R_k^T[tf, i] = sum_j xh[j, tf] * mask_k[j, i] via fp8
             DoubleRow matmuls (contraction 256 = 2 j-chunks per PE pass).
    stage 2: out[i, (t,o)] += lhs_k[tf_blk, i]^T @ thetap[k] over k, tf_blk
             in bf16 with block-diagonal padded Theta; ReLU on copy-out
             (split across Act+DVE so PSUM drains faster than PE refills).

Schedule: loads stream on SP in consumption order; per batch the kernel
runs stage-1 then stage-2 so b0's stage-2 covers b1's mask loads. Output
stores ride Pool's SWDGE queue for b0 and SP for b1 (SP's load stream is
done by then), two i-chunks per store.

Host pre-processing: premultiplied fp8 masks (power-of-2 scaled; scales
compensated inside thetap), fp8-swizzled xh, bf16 xht = attd*x (transposed),
diag(E) broadcast vector, block-diag thetap.
"""

import numpy as np

B, T, N, F_IN, F_OUT, K = 16, 12, 1024, 32, 64, 3
M_CORES = 8
NB = B // M_CORES          # batches per core
KM = 2                     # masked (non-diagonal) cheb paths: C1, E_off
P = 128                    # SBUF partitions
NJ = N // P                # 8 contraction chunks
NPAIR = NJ // 2            # 4 DoubleRow chunk pairs
TF = T * F_IN              # 384
NTFB = TF // P             # 3 tf blocks
TBLK = P // F_IN           # 4 t's per tf block
IS = 512                   # stage-1 strip width (1 PSUM bank of fp32)
NIS = N // IS              # 2 i strips
TO = TBLK * F_OUT          # 256 = stage-2 rhs width
TFO = T * F_OUT            # 768

_cache = {}


def _build(reps=1):
    import concourse.bacc as bacc
    import concourse.mybir as mybir
    import concourse.tile as tile

    DT = mybir.dt.bfloat16
    DT8 = mybir.dt.float8e4
    DTF = mybir.dt.float32
    DR = mybir.MatmulPerfMode.DoubleRow
    Relu = mybir.ActivationFunctionType.Relu

    nc = bacc.Bacc("TRN2", target_bir_lowering=False, debug=False)
    mask_d = nc.dram_tensor("mask8", [NB, KM, N, N], DT8, kind="ExternalInput")
    xh_d = nc.dram_tensor("xh8", [NB, P, NJ * TF], DT8, kind="ExternalInput")
    xht_d = nc.dram_tensor("xht", [NB, TF, N], DT, kind="ExternalInput")
    vec_d = nc.dram_tensor("gvec", [P, N], DT, kind="ExternalInput")
    thp_d = nc.dram_tensor("thetap", [K, P, TO], DT, kind="ExternalInput")
    # [b, i, t, o] layout: device stores are fully contiguous; host permutes
    # back to [b, t, i, o] afterwards.
    out_d = nc.dram_tensor("out", [NB, N, T, F_OUT], DT, kind="ExternalOutput")

    with tile.TileContext(nc) as tc:
        with (
            tc.tile_pool(name="mask", bufs=2 * KM) as mask_pool,
            tc.tile_pool(name="xh", bufs=2) as xh_pool,
            tc.tile_pool(name="xht", bufs=2) as xht_pool,
            tc.tile_pool(name="x2s", bufs=2) as x2s_pool,
            tc.tile_pool(name="rt", bufs=2) as rt_pool,
            tc.tile_pool(name="wts", bufs=1) as wts_pool,
            tc.tile_pool(name="osb", bufs=5) as out_pool,
            tc.tile_pool(name="rtps", bufs=2, space="PSUM") as rtps_pool,
            tc.tile_pool(name="outps", bufs=2, space="PSUM") as outps_pool,
        ):
            thp_sb = wts_pool.tile([P, K * TO], DT, tag="thp")
            vec_sb = wts_pool.tile([P, N], DT, tag="gvec")

            for rep in range(reps):
                first = rep == 0
                mask_tiles = {}
                xh_tiles = {}
                xht_tiles = {}

                def load_b(b):
                    # all input loads on SP in consumption order: its SEQ does
                    # nothing else, so issue order == transfer order.
                    xh_sb = xh_pool.tile([P, NJ * TF], DT8, tag="xh", name="xh_sb")
                    xh3d = xh_sb[:].rearrange("p (jb t) -> p jb t", jb=NJ)
                    xsrc = xh_d.ap()[b].rearrange("p (jb t) -> p jb t", jb=NJ)
                    nxh = 2 if b == 0 else 1
                    for c in range(nxh):
                        h = NJ // nxh
                        nc.sync.dma_start(
                            xh3d[:, c * h : (c + 1) * h, :],
                            xsrc[:, c * h : (c + 1) * h, :],
                        )
                    xh_tiles[b] = xh_sb

                    def load_mask(k, nch):
                        m_sb = mask_pool.tile(
                            [P, NJ * N], DT8, tag=f"mask{b}{k}", name=f"m{b}{k}"
                        )
                        msrc = mask_d.ap()[b][k].rearrange("(jb p) i -> p jb i", p=P)
                        mdst = m_sb[:].rearrange("p (jb i) -> p jb i", jb=NJ)
                        step = NJ // nch
                        for c in range(nch):
                            nc.sync.dma_start(
                                mdst[:, c * step : (c + 1) * step, :],
                                msrc[:, c * step : (c + 1) * step, :],
                            )
                        mask_tiles[b, k] = m_sb

                    load_mask(0, 4 if b == 0 else 2)
                    if b == 0 and first:
                        nc.sync.dma_start(vec_sb[:], vec_d.ap())
                        nc.sync.dma_start(
                            thp_sb[:].rearrange("p (k n) -> p k n", k=K),
                            thp_d.ap().rearrange("k p n -> p k n"),
                        )
                    xht_sb = xht_pool.tile([P, NTFB * N], DT, tag="xht", name="xht_sb")
                    xht3d = xht_sb[:].rearrange("p (c i) -> p c i", c=NTFB)
                    xhtsrc = xht_d.ap()[b].rearrange("(c p) i -> p c i", p=P)
                    if b == 0:
                        # b0 is latency-critical: interleave xht chunks with
                        # the k2 mask so stage-2's k0/diag operands land as
                        # they are consumed.
                        load_mask(1, 2)
                        for c in range(NTFB):
                            nc.sync.dma_start(xht3d[:, c : c + 1, :], xhtsrc[:, c : c + 1, :])
                    else:
                        nc.sync.dma_start(xht3d[:], xhtsrc[:])
                        load_mask(1, 2)
                    xht_tiles[b] = xht_sb

                load_b(0)
                load_b(1)

                for b in range(NB):
                    # ---- stage 1: R^T bf16 via fp8 DoubleRow matmuls ----
                    xh3 = xh_tiles[b][:].rearrange("p (jb t) -> p jb t", jb=NJ)
                    # diag(E) correction operand: xht * gvec, same [tf, i]
                    # layout as the stage-1 PSUM result.
                    x2s_sb = x2s_pool.tile([P, NTFB * N], DT, tag="x2s", name="x2s_sb")
                    for c in range(NTFB):
                        nc.vector.tensor_mul(
                            x2s_sb[:, c * N : (c + 1) * N],
                            xht_tiles[b][:, c * N : (c + 1) * N],
                            vec_sb[:],
                        )
                    rt_sb = rt_pool.tile([P, KM * NTFB * N], DT, tag="rt")
                    for k in range(KM):
                        m3 = mask_tiles[b, k][:].rearrange(
                            "p (jb i) -> p jb i", jb=NJ
                        )
                        for tfb in range(NTFB):
                            rtps = rtps_pool.tile([P, N], DTF, tag="rtps", name="rtps")
                            for q in range(NPAIR):
                                lhs = xh3[:, 2 * q : 2 * q + 2, tfb * P : (tfb + 1) * P]
                                for s in range(NIS):
                                    nc.tensor.matmul(
                                        rtps[:, s * IS : (s + 1) * IS],
                                        lhs,
                                        m3[:, 2 * q : 2 * q + 2, s * IS : (s + 1) * IS],
                                        start=(q == 0),
                                        stop=(q == NPAIR - 1),
                                        perf_mode=DR,
                                    )
                            base = (k * NTFB + tfb) * N
                            if k == 0:
                                nc.scalar.copy(rt_sb[:, base : base + N], rtps[:])
                            else:
                                # k=1 is the E path: add the bf16 diag(E)
                                # correction while copying out of PSUM.
                                nc.vector.tensor_add(
                                    rt_sb[:, base : base + N],
                                    rtps[:],
                                    x2s_sb[:, tfb * N : (tfb + 1) * N],
                                )

                    # ---- stage 2: out[i, (t,o)] over {diag, C1, E} ----
                    xht_sb = xht_tiles[b]
                    osb = None
                    for ic in range(NJ):
                        ops = outps_pool.tile([P, NTFB * TO], DTF, tag="ops", name="ops")
                        for tfb in range(NTFB):
                            for k in range(K):
                                if k == 0:
                                    lhs2 = xht_sb[:, tfb * N + ic * P : tfb * N + ic * P + P]
                                else:
                                    base = ((k - 1) * NTFB + tfb) * N + ic * P
                                    lhs2 = rt_sb[:, base : base + P]
                                nc.tensor.matmul(
                                    ops[:, tfb * TO : (tfb + 1) * TO],
                                    lhs2,
                                    thp_sb[:, k * TO : (k + 1) * TO],
                                    start=(k == 0),
                                    stop=(k == K - 1),
                                )
                        # ReLU split across both copy engines so the PSUM
                        # tile frees faster than PE refills it.
                        if ic % 2 == 0:
                            osb = out_pool.tile([P, 2 * TFO], DT, tag="osb")
                        half = (ic % 2) * TFO
                        nc.scalar.activation(
                            osb[:, half : half + TFO // 2], ops[:, : TFO // 2], Relu
                        )
                        nc.vector.tensor_relu(
                            osb[:, half + TFO // 2 : half + TFO], ops[:, TFO // 2 :]
                        )
                        if b == 1:
                            # single-chunk stores on SP (its load stream is
                            # done) for a fine-grained end-of-kernel drain.
                            nc.sync.dma_start(
                                out_d.ap()[b][ic * P : (ic + 1) * P, :, :]
                                .rearrange("p t o -> p (t o)"),
                                osb[:, half : half + TFO],
                            )
                        elif ic % 2 == 1:
                            # two i-chunks per store on Pool's SWDGE queue
                            # (keeps SP free while it streams loads).
                            pr = ic // 2
                            nc.gpsimd.dma_start(
                                out_d.ap()[b][pr * 2 * P : (pr + 1) * 2 * P, :, :]
                                .rearrange("(c p) t o -> p c (t o)", p=P),
                                osb[:].rearrange("p (c w) -> p c w", c=2),
                            )

    nc.compile()
    return nc


def _prep(x, att, cheb, Theta):
    """Host-side packing shared by kernel() and test harnesses.

    Returns the per-core ExternalInput dict list.
    """
    from ml_dtypes import bfloat16, float8_e4m3

    x = np.asarray(x, dtype=np.float32)
    att = np.asarray(att, dtype=np.float32)
    cheb = np.asarray(cheb, dtype=np.float32)
    Theta = np.asarray(Theta, dtype=np.float32)

    eye = np.eye(N, dtype=np.float32)
    C1 = cheb[1]
    E = cheb[2] + eye
    e = np.diagonal(E).copy()
    Eoff = E - np.diag(e)

    def pow2_scale(target, cur):
        return float(2.0 ** np.floor(np.log2(target / max(cur, 1e-30))))

    # premultiplied fp8 masks, transposed for stage 1 ([j, i] layout)
    m1 = C1[None, :, :] * att          # [B, i, j]
    m2 = Eoff[None, :, :] * att
    s1 = pow2_scale(100.0, np.abs(m1).max())
    s2 = pow2_scale(100.0, np.abs(m2).max())
    sx = pow2_scale(100.0, np.abs(x).max())
    mask8 = np.empty((B, KM, N, N), dtype=float8_e4m3)
    mask8[:, 0] = (m1 * s1).transpose(0, 2, 1).astype(float8_e4m3)
    mask8[:, 1] = (m2 * s2).transpose(0, 2, 1).astype(float8_e4m3)

    # xh fp8 pre-swizzled so each SBUF partition's row is contiguous in DRAM:
    # xh8[b, p, jb*TF + tf] = x[b, t, jb*128+p, f] * sx
    xh = (x * sx).transpose(0, 2, 1, 3).reshape(B, N, TF)
    xh8 = np.ascontiguousarray(
        xh.reshape(B, NJ, P, TF).transpose(0, 2, 1, 3).reshape(B, P, NJ * TF)
    ).astype(float8_e4m3)

    # diagonal path: xht[b, tf, i] = x[b,t,i,f] * att[b,i,i], bf16
    attd = np.einsum("bii->bi", att)
    xht = np.ascontiguousarray(
        (x * attd[:, None, :, None]).transpose(0, 1, 3, 2).reshape(B, TF, N)
    ).astype(bfloat16)

    # diag(E) broadcast vector in stage-1 scaled units
    gvec = np.broadcast_to((e * s2 * sx).astype(bfloat16)[None, :], (P, N))
    gvec = np.ascontiguousarray(gvec)

    # block-diag thetap; per-slot scale compensation for the fp8 paths
    Th = np.stack([Theta[0] - Theta[2], Theta[1] / (s1 * sx), Theta[2] / (s2 * sx)])
    thetap = np.zeros((K, P, TO), dtype=np.float32)
    for tr in range(TBLK):
        thetap[:, tr * F_IN : (tr + 1) * F_IN, tr * F_OUT : (tr + 1) * F_OUT] = Th
    thetap = thetap.astype(bfloat16)

    return [
        {
            "mask8": mask8[c * NB : (c + 1) * NB],
            "xh8": xh8[c * NB : (c + 1) * NB],
            "xht": xht[c * NB : (c + 1) * NB],
            "gvec": gvec,
            "thetap": thetap,
        }
        for c in range(M_CORES)
    ]


def kernel(x, spatial_attention, cheb, Theta):
    from concourse.bass_utils import run_bass_kernel_spmd

    cheb = np.asarray(cheb, dtype=np.float32)
    if np.abs(cheb[0] - np.eye(N, dtype=np.float32)).max() > 1e-5:
        # T0 of any Chebyshev basis is the identity; the device kernel
        # hardcodes that. Unreachable for this problem's generator — kept
        # only so a different basis still returns the right answer.
        x = np.asarray(x, dtype=np.float32)
        att = np.asarray(spatial_attention, dtype=np.float32)
        Theta = np.asarray(Theta, dtype=np.float32)
        tk = cheb[None] * att[:, None]
        rhs = np.einsum("bkij,btjf->btkif", tk, x)
        return np.maximum(np.einsum("btkif,kfo->btio", rhs, Theta), 0.0)

    if "k" not in _cache:
        _cache["k"] = _build()
    nc = _cache["k"]

    in_maps = _prep(x, spatial_attention, cheb, Theta)
    try:
        res = run_bass_kernel_spmd(nc, in_maps, list(range(M_CORES)))
    except Exception:
        # transient NRT device hiccups recover on redispatch
        res = run_bass_kernel_spmd(nc, in_maps, list(range(M_CORES)))
    out = np.concatenate(
        [np.asarray(res.results[c]["out"]) for c in range(M_CORES)], axis=0
    )
    # device layout is [b, i, t, o] bf16 -> [b, t, i, o] fp32
    return np.ascontiguousarray(out.transpose(0, 2, 1, 3)).astype(np.float32)
